# revision 2
# baseline (speedup 1.0000x reference)
"""Shifted-window cross-attention, data-parallel over batch B=8 across
8 NeuronCores (one image per core; CxC weights replicated).

kernel(**inputs) takes FULL unsharded inputs (x, y: [8,224,224,192] f32;
Wq/Wk/Wv/Wo: [192,192] f32) and returns the FULL output tuple
(out [8,224,224,192] f32, att [8192,6,49,49] f32).
"""
import numpy as np

WS = 7
SHIFT = 3
HEADS = 6
B, H, W, C = 8, 224, 224, 192
N = WS * WS
D = C // HEADS
NW = (H // WS) * (W // WS)  # windows per image


def _per_image(x, y, Wq, Wk, Wv, Wo):
    """x, y: [H, W, C] one image -> (out [H,W,C], att [NW,HEADS,N,N]).

    Same math as the reference, written with batched matmuls and a
    hand-rolled softmax (logits are bounded, so no max-subtraction is
    needed and the result is identical up to fp rounding).
    """
    import jax.numpy as jnp
    ws, shift, h, d, n = WS, SHIFT, HEADS, D, N

    mask = jnp.where(x > 0.95, 0.0, 1.0).mean(axis=-1, keepdims=True)
    x = x * mask

    x = jnp.roll(x, shift=(-shift, -shift), axis=(0, 1))
    y = jnp.roll(y, shift=(-shift, -shift), axis=(0, 1))
    mask = jnp.roll(mask, shift=(-shift, -shift), axis=(0, 1))

    def part(t):
        c = t.shape[-1]
        t = t.reshape(H // ws, ws, W // ws, ws, c)
        return t.transpose(0, 2, 1, 3, 4).reshape(-1, ws * ws, c)

    xw = part(x)                       # [nW, N, C]
    yw = part(y)
    mw = part(mask)[..., 0]            # [nW, N]

    nW = xw.shape[0]
    q = (xw @ Wq).reshape(nW, n, h, d).transpose(0, 2, 1, 3)  # [nW,h,N,d]
    k = (yw @ Wk).reshape(nW, n, h, d).transpose(0, 2, 1, 3)
    v = (yw @ Wv).reshape(nW, n, h, d).transpose(0, 2, 1, 3)

    scale = 1.0 / np.sqrt(np.float32(d))
    logits = (q * scale) @ k.transpose(0, 1, 3, 2)            # [nW,h,N,N]
    logits = logits + jnp.log(mw + 1e-6)[:, None, None, :]
    e = jnp.exp(logits)
    att = e / jnp.sum(e, axis=-1, keepdims=True)

    o = (att @ v).transpose(0, 2, 1, 3).reshape(nW, n, C) @ Wo
    o = o.reshape(H // ws, W // ws, ws, ws, C)
    o = o.transpose(0, 2, 1, 3, 4).reshape(H, W, C)
    o = jnp.roll(o, shift=(shift, shift), axis=(0, 1))
    return o, att


def _run_neuron(x, y, Wq, Wk, Wv, Wo):
    import jax
    devs = [dv for dv in jax.devices() if dv.platform != 'cpu'][:8]
    if len(devs) < 8:
        raise RuntimeError('need 8 accelerator cores')
    fn = jax.pmap(_per_image, in_axes=(0, 0, None, None, None, None),
                  devices=devs)
    out, att = fn(x, y, Wq, Wk, Wv, Wo)
    out = np.asarray(out, dtype=np.float32)
    att = np.asarray(att, dtype=np.float32).reshape(B * NW, HEADS, N, N)
    return out, att


def _run_cpu(x, y, Wq, Wk, Wv, Wo):
    import jax
    cpu = jax.devices('cpu')[0]
    with jax.default_device(cpu):
        pf = jax.jit(_per_image, backend='cpu')
        outs, atts = [], []
        for b in range(B):
            o, a = pf(x[b], y[b], Wq, Wk, Wv, Wo)
            outs.append(np.asarray(o))
            atts.append(np.asarray(a))
    out = np.stack(outs).astype(np.float32)
    att = np.concatenate(atts).astype(np.float32).reshape(
        B * NW, HEADS, N, N)
    return out, att


def kernel(x, y, Wq, Wk, Wv, Wo):
    import os
    import signal
    x = np.ascontiguousarray(x, dtype=np.float32)
    y = np.ascontiguousarray(y, dtype=np.float32)
    Wq = np.asarray(Wq, np.float32)
    Wk = np.asarray(Wk, np.float32)
    Wv = np.asarray(Wv, np.float32)
    Wo = np.asarray(Wo, np.float32)

    if os.environ.get('KERNEL_FORCE_CPU'):
        return _run_cpu(x, y, Wq, Wk, Wv, Wo)

    # Neuron path, guarded by an alarm so a wedged compile can't hang
    # the caller; fall back to the CPU implementation on any failure.
    timeout = int(os.environ.get('KERNEL_NEURON_TIMEOUT', '600'))

    class _Timeout(Exception):
        pass

    def _raise(*_):
        raise _Timeout()

    old = None
    try:
        old = signal.signal(signal.SIGALRM, _raise)
        signal.alarm(timeout)
        result = _run_neuron(x, y, Wq, Wk, Wv, Wo)
        signal.alarm(0)
        return result
    except BaseException:
        signal.alarm(0)
        return _run_cpu(x, y, Wq, Wk, Wv, Wo)
    finally:
        signal.alarm(0)
        if old is not None:
            signal.signal(signal.SIGALRM, old)
